# revision 36
# baseline (speedup 1.0000x reference)
"""MHA kernel, variant X: paired scores + ones-column softmax denominator.

Same sharding/host-prep as kernel.py.  Differences from the den-matmul
variant: v carries a 65th ones column per head (softmax denominator rides
row 64 of the [65,512] AV accumulators), so there are no denominator
matmuls and the projection PSUM bank is double-buffered; the score
matmuls of the head pair (2t, 2t+1) are still emitted adjacently on
disjoint PE row halves so they co-run; background projections are
priority-demoted so they fill idle PE slots without splitting pairs.
"""

import numpy as np
import ml_dtypes

import concourse.bacc as bacc
import concourse.bass as bass
import concourse.mybir as mybir
import concourse.tile as tile
from concourse.bass_utils import run_bass_kernel_spmd

B, S, D, H = 4, 2048, 1024, 16
DH = D // H          # 64
HG = H // 2          # 8 heads per core
DG = HG * DH         # 512 dims per core
N_CORES = 8
VBLK = DH + 1        # 65: v columns + ones column

BF16 = mybir.dt.bfloat16
F32 = mybir.dt.float32

ST = S // 128
QB = S // 512
KT = D // 128
AF = mybir.ActivationFunctionType
ALU = mybir.AluOpType

from contextlib import contextmanager


_DEMOTE = True


@contextmanager
def _bg_prio(tc, off=1_000_000):
    if not _DEMOTE:
        yield
        return
    tc.cur_priority += off
    try:
        yield
    finally:
        tc.cur_priority -= off


def build_program(loop_r=0):
    nc = bacc.Bacc("TRN2", target_bir_lowering=False, debug=False,
                   num_devices=N_CORES)

    xq = nc.declare_dram_parameter("xq", [D, S], BF16, isOutput=False)
    xk = nc.declare_dram_parameter("xk", [D, S], BF16, isOutput=False)
    xv = nc.declare_dram_parameter("xv", [D, S], BF16, isOutput=False)
    wq = nc.declare_dram_parameter("wq", [D, DG], BF16, isOutput=False)
    wk = nc.declare_dram_parameter("wk", [D, DG], BF16, isOutput=False)
    wv = nc.declare_dram_parameter("wv", [D, DG], BF16, isOutput=False)
    wo = nc.declare_dram_parameter("wo", [DG, D], BF16, isOutput=False)
    bq = nc.declare_dram_parameter("bq", [DG, 1], F32, isOutput=False)
    bk = nc.declare_dram_parameter("bk", [DG, 1], F32, isOutput=False)
    bv = nc.declare_dram_parameter("bv", [1, DG], F32, isOutput=False)
    out = nc.declare_dram_parameter("out", [S, D], F32, isOutput=True)

    with tile.TileContext(nc) as tc:
        with (
            tc.tile_pool(name="persist", bufs=1) as persist,
            tc.tile_pool(name="xin", bufs=4) as xin,
            tc.tile_pool(name="xvin", bufs=1) as xvin,
            tc.tile_pool(name="exp", bufs=8) as expp,
            tc.tile_pool(name="small", bufs=3) as small,
            tc.tile_pool(name="outp", bufs=3) as outp,
            tc.tile_pool(name="pssc", bufs=2, space="PSUM") as pssc,
            tc.tile_pool(name="psav", bufs=2, space="PSUM") as psav,
            tc.tile_pool(name="ps512", bufs=2, space="PSUM") as ps512,
        ):
            import contextlib
            loop_cm = tc.For_i(0, loop_r, 1) if loop_r else contextlib.nullcontext()
            with loop_cm:
                emit_body(nc, tc, locals())
    nc.compile()
    return nc


def emit_body(nc, tc, pools):
    persist = pools["persist"]; xin = pools["xin"]; xvin = pools["xvin"]
    expp = pools["expp"]; small = pools["small"]; outp = pools["outp"]
    pssc = pools["pssc"]; psav = pools["psav"]; ps512 = pools["ps512"]
    xq = pools["xq"]; xk = pools["xk"]; xv = pools["xv"]
    wq = pools["wq"]; wk = pools["wk"]; wv = pools["wv"]; wo = pools["wo"]
    bq = pools["bq"]; bk = pools["bk"]; bv = pools["bv"]; out = pools["out"]

    wq_sb = persist.tile([128, KT * DG], BF16, tag="wq")
    wk_sb = persist.tile([128, KT * DG], BF16, tag="wk")
    wv_sb = persist.tile([128, KT * DG], BF16, tag="wv")
    wo_sb = persist.tile([128, 4 * D], BF16, tag="wo")
    bq_sb = persist.tile([128, 4], F32, tag="bq")
    bk_sb = persist.tile([128, 4], F32, tag="bk")
    bv_row = persist.tile([1, DG], F32, tag="bvr")
    bvb = persist.tile([128, DG], F32, tag="bvb")
    ones128 = persist.tile([1, 128], F32, tag="ones128")
    nc.gpsimd.memset(ones128[:], 1.0)
    # PE warmup through the initial DMA wait (keeps HAM un-throttled)
    wmt = persist.tile([128, 256], BF16, tag="wmt")
    nc.gpsimd.memset(wmt[:], 0.25)
    psw = pssc.tile([128, 1024], F32, tag="sc", name="psw")
    for _ in range(30):
        nc.tensor.matmul(psw[0:16, 0:256], wmt[:, 0:16], wmt[:],
                         start=True, stop=True)
    nc.sync.dma_start(bv_row[:], bv[:])
    ps_bv = pssc.tile([128, 1024], F32, tag="sc", name="ps_bv")
    nc.tensor.matmul(ps_bv[:, 0:512], ones128[:], bv_row[:], start=True, stop=True)
    nc.vector.tensor_copy(bvb[:], ps_bv[:, 0:512])
    bvb_view = bvb[:].rearrange("p (h c) -> p h c", c=DH)
    nc.sync.dma_start(
        bk_sb[:].rearrange("p (t o) -> p t o", o=1),
        bk[:].rearrange("(t p) o -> p t o", p=128),
    )

    qt = [persist.tile([128, S], BF16, tag=f"qt{t}", name=f"qt{t}") for t in range(4)]
    kt = [persist.tile([128, S], BF16, tag=f"kt{t}", name=f"kt{t}") for t in range(4)]
    v_sb = persist.tile([128, ST * HG * VBLK], BF16, tag="v_sb")
    ao = [persist.tile([128, S], BF16, tag=f"ao{t}", name=f"ao{t}") for t in range(4)]

    v_view = v_sb[:].rearrange("p (s h c) -> p s h c", s=ST, h=HG, c=VBLK)
    nc.gpsimd.memset(v_view[:, :, :, DH : DH + 1], 1.0)

    chunk_tiles = {}

    def prefetch_chunk(n):
        xk_sb = xin.tile([128, KT * 512], BF16, tag="xkq", name=f"xk_sb{n}")
        nc.sync.dma_start(
            xk_sb[:].rearrange("p (j c) -> p j c", j=KT),
            xk[:, bass.ts(n, 512)].rearrange("(j p) c -> p j c", p=128),
        )
        xq_sb = xin.tile([128, KT * 512], BF16, tag="xkq", name=f"xq_sb{n}")
        nc.sync.dma_start(
            xq_sb[:].rearrange("p (j c) -> p j c", j=KT),
            xq[:, bass.ts(n, 512)].rearrange("(j p) c -> p j c", p=128),
        )
        chunk_tiles[n] = (xk_sb, xq_sb)

    nc.sync.dma_start(
        wk_sb[:].rearrange("p (j c) -> p j c", j=KT),
        wk[:].rearrange("(j p) c -> p j c", p=128),
    )
    prefetch_chunk(0)
    nc.sync.dma_start(
        wq_sb[:].rearrange("p (j c) -> p j c", j=KT),
        wq[:].rearrange("(j p) c -> p j c", p=128),
    )
    nc.sync.dma_start(
        bq_sb[:].rearrange("p (t o) -> p t o", o=1),
        bq[:].rearrange("(t p) o -> p t o", p=128),
    )
    nc.sync.dma_start(
        wv_sb[:].rearrange("p (j c) -> p j c", j=KT),
        wv[:].rearrange("(j p) c -> p j c", p=128),
    )
    xv_sb = xvin.tile([128, KT * S], BF16, tag="xv")
    xv_view = xv_sb[:].rearrange("p (j c) -> p j c", j=KT)

    def fetch_xv_chunk(cchunk):
        nc.sync.dma_start(
            xv_view[:, :, 512 * cchunk : 512 * (cchunk + 1)],
            xv[:, bass.ts(cchunk, 512)].rearrange("(j p) c -> p j c", p=128),
        )

    fetch_xv_chunk(0)
    prefetch_chunk(1)
    fetch_xv_chunk(1)
    xv_t = [xv_sb[:, bass.ts(j, S)] for j in range(KT)]
    nc.sync.dma_start(
        wo_sb[:].rearrange("p (j c) -> p j c", j=4),
        wo[:].rearrange("(j p) c -> p j c", p=128),
    )
    fetch_xv_chunk(2)
    fetch_xv_chunk(3)

    def emit_kproj(n, t):
        xk_sb, _ = chunk_tiles[n]
        ps = ps512.tile([128, 512], F32, tag="mm512", name="psk")
        for j in range(KT):
            nc.tensor.matmul(
                ps[:],
                wk_sb[:, j * DG + t * 128 : j * DG + (t + 1) * 128],
                xk_sb[:, bass.ts(j, 512)],
                start=(j == 0),
                stop=(j == KT - 1),
            )
        nc.vector.tensor_scalar_add(
            kt[t][:, bass.ts(n, 512)], ps[:], bk_sb[:, t : t + 1]
        )

    def emit_qproj(n, t):
        _, xq_sb = chunk_tiles[n]
        ps = ps512.tile([128, 512], F32, tag="mm512", name="psq")
        for j in range(KT):
            nc.tensor.matmul(
                ps[:],
                wq_sb[:, j * DG + t * 128 : j * DG + (t + 1) * 128],
                xq_sb[:, bass.ts(j, 512)],
                start=(j == 0),
                stop=(j == KT - 1),
            )
        nc.vector.tensor_scalar(
            qt[t][:, bass.ts(n, 512)], ps[:],
            bq_sb[:, t : t + 1], 0.125, ALU.add, ALU.mult,
        )

    def emit_vproj(s):
        ps = ps512.tile([128, 512], F32, tag="mm512", name="psv")
        for j in range(KT):
            nc.tensor.matmul(
                ps[:],
                xv_t[j][:, bass.ts(s, 128)],
                wv_sb[:, bass.ts(j, DG)],
                start=(j == 0),
                stop=(j == KT - 1),
            )
        nc.vector.tensor_add(
            v_view[:, s, :, 0:DH],
            ps[:].rearrange("p (h c) -> p h c", c=DH),
            bvb_view,
        )

    def emit_oproj_m(s, m, drain="vector"):
        po = ps512.tile([128, 512], F32, tag="mm512", name="po")
        for kk in range(4):
            nc.tensor.matmul(
                po[:],
                ao[kk][:, bass.ts(s, 128)],
                wo_sb[:, kk * D + m * 512 : kk * D + (m + 1) * 512],
                start=(kk == 0),
                stop=(kk == 3),
            )
        ob = outp.tile([128, 512], F32, tag="ob", name="ob")
        if drain == "scalar":
            nc.scalar.copy(ob[:], po[:])
        else:
            nc.vector.tensor_copy(ob[:], po[:])
        nc.sync.dma_start(out[bass.ts(s, 128), bass.ts(m, 512)], ob[:])

    for t in range(4):
        emit_kproj(0, t)
    for t in range(4):
        emit_qproj(0, t)

    def v_ap(j, t, r):
        h = 2 * t + r
        return v_sb[:, (j * HG + h) * VBLK : (j * HG + h) * VBLK + VBLK]

    for n in range(QB):
        if n + 1 < QB:
            if n >= 1:
                prefetch_chunk(n + 1)
            bg = (
                [lambda t=t: emit_kproj(n + 1, t) for t in range(4)]
                + [lambda t=t: emit_qproj(n + 1, t) for t in range(4)]
                + [lambda s=s: emit_vproj(s) for s in range(4 * n + 4, 4 * n + 8)]
            )
        else:
            bg = []
        if n == 0:
            bg = [lambda s=s: emit_vproj(s) for s in range(0, 4)] + bg
        elif n == 1:
            bg += [lambda s=s, m=m: emit_oproj_m(s, m)
                   for s in range(0, 4) for m in range(2)]
        elif n == 3:
            bg += [lambda s=s, m=m: emit_oproj_m(s, m)
                   for s in range(4, 12) for m in range(2)]

        nk = 4 * (n + 1)

        # spread this block's bg entries evenly over its 4 t-iterations
        _pop_quota = {}
        _rem = len(bg)
        for _t in range(4):
            q = (len(bg) * (_t + 1)) // 4 - (len(bg) * _t) // 4
            _pop_quota[(n, _t)] = q

        for t in range(4):
            qA = qt[t][0:64, bass.ts(n, 512)]
            qB = qt[t][64:128, bass.ts(n, 512)]
            avA = psav.tile([VBLK, 512], F32, tag="av", name=f"avA{t}")
            avB = psav.tile([VBLK, 512], F32, tag="av", name=f"avB{t}")

            def make_full(j):
                ex_box = []

                def s1():
                    sc = pssc.tile([128, 1024], F32, tag="sc", name="sc")
                    nc.tensor.matmul(sc[:, 0:512],
                                     kt[t][0:64, bass.ts(j, 128)], qA,
                                     start=True, stop=True)
                    nc.tensor.matmul(sc[:, 512:1024],
                                     kt[t][64:128, bass.ts(j, 128)], qB,
                                     start=True, stop=True)
                    ex = expp.tile([128, 1024], BF16, tag="ex", name="ex")
                    nc.scalar.activation(ex[:], sc[:], AF.Exp)
                    ex_box.append(ex)

                def s2():
                    ex = ex_box[0]
                    nc.tensor.matmul(avA[:], v_ap(j, t, 0), ex[:, 0:512],
                                     start=(j == 0), stop=(j == nk - 1))
                    nc.tensor.matmul(avB[:], v_ap(j, t, 1), ex[:, 512:1024],
                                     start=(j == 0), stop=(j == nk - 1))

                return s1, s2

            def make_band(rp, last):
                r0, r1 = 2 * rp, 2 * rp + 1
                nw0, nw1 = 512 - 128 * r0, 512 - 128 * r1
                j0, j1 = 4 * n + r0, 4 * n + r1
                ex_box = []

                def s1():
                    scA = pssc.tile([128, 1024], F32, tag="sc", name="scbA")
                    scB = pssc.tile([128, 1024], F32, tag="sc", name="scbB")
                    for (jj, off, nw, ri) in ((j0, 0, nw0, r0), (j1, nw0, nw1, r1)):
                        nc.tensor.matmul(
                            scA[:, off : off + nw],
                            kt[t][0:64, bass.ts(jj, 128)],
                            qA[:, 128 * ri : 512],
                            start=True, stop=True)
                        nc.tensor.matmul(
                            scB[:, off : off + nw],
                            kt[t][64:128, bass.ts(jj, 128)],
                            qB[:, 128 * ri : 512],
                            start=True, stop=True)
                    exA = expp.tile([128, 1024], BF16, tag="ex", name="exbA")
                    exB = expp.tile([128, 1024], BF16, tag="ex", name="exbB")
                    nc.scalar.activation(exA[:, 0 : nw0 + nw1],
                                         scA[:, 0 : nw0 + nw1], AF.Exp)
                    nc.scalar.activation(exB[:, 0 : nw0 + nw1],
                                         scB[:, 0 : nw0 + nw1], AF.Exp)
                    for exx in (exA, exB):
                        for off in (0, nw0):
                            nc.gpsimd.affine_select(
                                out=exx[:, off : off + 128],
                                in_=exx[:, off : off + 128],
                                compare_op=ALU.is_ge,
                                fill=0.0,
                                base=0,
                                pattern=[[1, 128]],
                                channel_multiplier=-1,
                            )
                    ex_box.append(exA)
                    ex_box.append(exB)

                def s2():
                    exA, exB = ex_box
                    for (jj, off, nw, ri) in ((j0, 0, nw0, r0), (j1, nw0, nw1, r1)):
                        nc.tensor.matmul(
                            avA[:, 128 * ri : 512], v_ap(jj, t, 0),
                            exA[:, off : off + nw],
                            start=(jj == 0), stop=(jj == nk - 1))
                    if last:
                        fin1(avA, 0)
                    for (jj, off, nw, ri) in ((j0, 0, nw0, r0), (j1, nw0, nw1, r1)):
                        nc.tensor.matmul(
                            avB[:, 128 * ri : 512], v_ap(jj, t, 1),
                            exB[:, off : off + nw],
                            start=(jj == 0), stop=(jj == nk - 1))

                return s1, s2

            rb_box = {}

            def fin1(av_, r):
                den = small.tile([1, 512], F32, tag="den", name="den")
                nc.vector.tensor_copy(den[:], av_[DH : DH + 1, :])
                recip = small.tile([1, 512], F32, tag="recip", name="recip")
                nc.vector.reciprocal_approx_fast(recip[:], den[:])
                rb = small.tile([DH, 512], F32, tag="rb", name="rb")
                nc.gpsimd.partition_broadcast(rb[:], recip[:], channels=DH)
                rb_box[r] = rb

            def fin2(av_, r):
                dst = ao[t][r * DH : (r + 1) * DH, bass.ts(n, 512)]
                nc.vector.tensor_mul(dst, av_[0:DH, :], rb_box[r][:])

            units = []
            for j in range(4 * n):
                units.append(make_full(j))
            for rp in range(2):
                units.append(make_band(rp, last=(rp == 1)))

            prev = None
            nu = len(units)
            quota = _pop_quota.get((n, t), len(bg))
            for ui, (s1, s2) in enumerate(units):
                s1()
                npop = (quota * (ui + 1)) // nu - (quota * ui) // nu
                for _ in range(npop):
                    if bg:
                        with _bg_prio(tc):
                            bg.pop(0)()
                if prev is not None:
                    prev()
                prev = s2
            prev()

            fin2(avA, 0)
            fin1(avB, 1)
            fin2(avB, 1)
            if bg:
                with _bg_prio(tc):
                    bg.pop(0)()
        while bg:
            with _bg_prio(tc):
                bg.pop(0)()
        if n == QB - 1:
            for s in range(4 * n, 4 * n + 4):
                for m in range(2):
                    emit_oproj_m(s, m, drain=("scalar" if m else "vector"))


_NC = None


def _get_program():
    global _NC
    if _NC is None:
        _NC = build_program()
    return _NC


def make_in_maps(query, key, value, Wq, bq, Wk, bk, Wv, bv, Wo):
    bf = ml_dtypes.bfloat16
    in_maps = []
    xqs = [np.ascontiguousarray(query[b].T).astype(bf) for b in range(B)]
    xks = [np.ascontiguousarray(key[b].T).astype(bf) for b in range(B)]
    xvs = [np.ascontiguousarray(value[b].T).astype(bf) for b in range(B)]
    for c in range(N_CORES):
        b, hg = c // 2, c % 2
        sl = slice(hg * DG, (hg + 1) * DG)
        in_maps.append({
            "xq": xqs[b], "xk": xks[b], "xv": xvs[b],
            "wq": np.ascontiguousarray(Wq[sl, :].T).astype(bf),
            "wk": np.ascontiguousarray(Wk[sl, :].T).astype(bf),
            "wv": np.ascontiguousarray(Wv[sl, :].T).astype(bf),
            "wo": np.ascontiguousarray(Wo[:, sl].T).astype(bf),
            "bq": np.asarray(bq[sl], np.float32).reshape(DG, 1),
            "bk": np.asarray(bk[sl], np.float32).reshape(DG, 1),
            "bv": np.asarray(bv[sl], np.float32).reshape(1, DG),
        })
    return in_maps


def combine_outputs(results, bo):
    out = np.empty((B, S, D), np.float32)
    for b in range(B):
        out[b] = results[2 * b]["out"] + results[2 * b + 1]["out"]
        out[b] += np.asarray(bo, np.float32)[None, :]
    return out


def kernel(query, key, value, mask, Wq, bq, Wk, bk, Wv, bv, Wo, bo):
    nc = _get_program()
    in_maps = make_in_maps(
        np.asarray(query, np.float32), np.asarray(key, np.float32),
        np.asarray(value, np.float32), np.asarray(Wq, np.float32),
        np.asarray(bq, np.float32), np.asarray(Wk, np.float32),
        np.asarray(bk, np.float32), np.asarray(Wv, np.float32),
        np.asarray(bv, np.float32), np.asarray(Wo, np.float32),
    )
    res = run_bass_kernel_spmd(nc, in_maps, list(range(N_CORES)))
    return combine_outputs(res.results, np.asarray(bo, np.float32))
